# revision 33
# baseline (speedup 1.0000x reference)
"""Causal multi-head attention (B=4, S=2048, D=1024, H=16) on 8 NeuronCores.

Sharding: core c = (batch b = c//2, head-group hg = c%2). Each core computes
8 heads of one batch: QKV projection (bf16 matmuls), causal flash-style
attention (bf16 matmuls, exp-without-max softmax with a ones-column
denominator), and a row-parallel out-projection partial. Host sums the two
bf16 head-group partials per batch, adds bias, and transposes.

All HBM payloads are bf16 (host pre-casts inputs, partials come back bf16).
Layouts are feature-major ([feature, token]) except v (token-major) so
attn@v needs no transposes. Head pairs are packed into PE row groups
(rows 0-63 / 64-127); the two K=64 score matmuls run concurrently on PE
row-tiles T0/T8. PSUM score tiles are 2 banks wide (even head in columns
0-511, odd in 512-1023) so one ACT exp covers both heads. Out-projection
runs K=128 matmuls accumulating all 4 head pairs in PSUM. Emission
interleaves QKV feature-groups, attention blocks, and out-proj blocks so
the single shared PSUM pool pipelines across phases.
"""
import numpy as np
from contextlib import ExitStack

import ml_dtypes

B, S, D, H = 4, 2048, 1024, 16
HD = 64            # head dim
HPC = 8            # heads per core
F = HPC * HD       # 512 features per head-group
QT = 512           # q tile (free dim)
NQI = S // QT      # 4
NKT = S // 128     # 16
NDK = D // 128     # 8 contraction tiles for projections
SCALE = HD ** -0.5

_CACHE = {}


def _build():
    import concourse.bacc as bacc
    import concourse.tile as tile
    import concourse.mybir as mybir

    f32 = mybir.dt.float32
    bf16 = mybir.dt.bfloat16
    EXP = mybir.ActivationFunctionType.Exp

    nc = bacc.Bacc("TRN2", target_bir_lowering=False, debug=False)
    xT = nc.dram_tensor("xT", [D, S], bf16, kind="ExternalInput").ap()
    w_sl = nc.dram_tensor("w_sl", [D, 3 * F], bf16, kind="ExternalInput").ap()
    wo_sl = nc.dram_tensor("wo_sl", [F, D], bf16, kind="ExternalInput").ap()
    mask2 = nc.dram_tensor("mask2", [128, 256], bf16, kind="ExternalInput").ap()
    out = nc.dram_tensor("out", [D, S], bf16, kind="ExternalOutput").ap()

    with tile.TileContext(nc) as tc:
        with ExitStack() as ctx:
            # ---- SBUF pools that must not overlap the x/w region ----
            misc = ctx.enter_context(tc.tile_pool(name="misc", bufs=1))
            mask_sb = misc.tile([128, 256], bf16, name="mask_sb", tag="mask")
            nc.sync.dma_start(mask_sb[:], mask2)

            pqk = ctx.enter_context(tc.tile_pool(name="pqk", bufs=1))
            pv = ctx.enter_context(tc.tile_pool(name="pv", bufs=1))
            patt = ctx.enter_context(tc.tile_pool(name="patt", bufs=16))
            pP = ctx.enter_context(tc.tile_pool(name="pP", bufs=3))
            pr = ctx.enter_context(tc.tile_pool(name="pr", bufs=2))
            prr = ctx.enter_context(tc.tile_pool(name="prr", bufs=1))
            pwo = ctx.enter_context(tc.tile_pool(name="pwo", bufs=1))
            pstg = ctx.enter_context(tc.tile_pool(name="pstg", bufs=3))

            # wo tiles created here; their DMAs are emitted after the x
            # loads so they don't delay the prologue-critical tensors
            wo_t = [pwo.tile([128, D], bf16, name=f"wo{g}", tag=f"wo{g}")
                    for g in range(4)]

            q_sb = [pqk.tile([128, S], bf16, name=f"q{g}", tag=f"q{g}")
                    for g in range(4)]
            k_sb = [pqk.tile([128, S], bf16, name=f"k{g}", tag=f"k{g}")
                    for g in range(4)]
            v_sb = [pv.tile([128, HPC * (HD + 1)], bf16, name=f"v{t}",
                            tag=f"v{t}") for t in range(NKT)]

            psum = ctx.enter_context(
                tc.tile_pool(name="psum", bufs=2, space="PSUM"))

            att_m = {}
            opq = []

            def op_unit(qi, dt):
                def emit():
                    dcol = slice(dt * 128, dt * 128 + 128)
                    ps = psum.tile([128, QT], f32,
                                   name=f"op{dt}{qi}", tag="big")
                    for pg in range(4):
                        nc.tensor.matmul(
                            ps[:], wo_t[pg][:, dcol], att_m[(pg, qi)][:],
                            start=(pg == 0), stop=(pg == 3))
                    s2 = pstg.tile([128, QT], bf16, name=f"s2{dt}{qi}",
                                   tag="s2")
                    nc.vector.tensor_copy(s2[:], ps[:])
                    nc.sync.dma_start(
                        out[dt * 128:(dt + 1) * 128,
                            qi * QT:(qi + 1) * QT], s2[:])
                return emit

            def attn_block(pg, qi):
                """Scores + exp + attn@v + normalize for head pair pg,
                q-range [qi*QT, (qi+1)*QT)."""
                nkt = 4 * qi + 4
                qs = qi * QT
                he, ho = 2 * pg, 2 * pg + 1
                C = HD + 1
                ao = psum.tile([HD + 1, 2 * QT], f32,
                               name=f"ao{pg}{qi}", tag="ao")
                for kt in range(nkt):
                    d = kt - 4 * qi
                    n0 = 0 if d < 0 else 128 * d
                    kcol = slice(kt * 128, kt * 128 + 128)
                    sc = psum.tile([128, 2 * QT], f32,
                                   name=f"sc{pg}{qi}{kt}", tag="big")
                    nc.tensor.matmul(
                        sc[:, n0:QT], k_sb[pg][0:64, kcol],
                        q_sb[pg][0:64, qs + n0:qs + QT],
                        start=True, stop=True)
                    nc.tensor.matmul(
                        sc[:, QT + n0:2 * QT], k_sb[pg][64:128, kcol],
                        q_sb[pg][64:128, qs + n0:qs + QT],
                        start=True, stop=True)
                    pt = pP.tile([128, 2 * QT], bf16,
                                 name=f"pt{pg}{qi}{kt}", tag="P")
                    sc3 = sc.rearrange("p (h c) -> p h c", h=2)
                    pt3 = pt.rearrange("p (h c) -> p h c", h=2)
                    nc.scalar.activation(pt3[:, :, n0:QT], sc3[:, :, n0:QT],
                                         EXP, scale=SCALE)
                    if d >= 0:
                        m3 = mask_sb.rearrange("p (h c) -> p h c", h=2)
                        nc.vector.tensor_mul(pt3[:, :, n0:n0 + 128],
                                             pt3[:, :, n0:n0 + 128], m3[:])
                    if kt % 2 == 1 and opq:
                        opq.pop(0)()
                    st = (kt == 0)
                    sp = (kt == nkt - 1)
                    nc.tensor.matmul(
                        ao[:, n0:QT], v_sb[kt][:, he * C:(he + 1) * C],
                        pt[:, n0:QT], start=st, stop=sp)
                    nc.tensor.matmul(
                        ao[:, QT + n0:2 * QT],
                        v_sb[kt][:, ho * C:(ho + 1) * C],
                        pt[:, QT + n0:2 * QT], start=st, stop=sp)

                # normalize: 1/rowsum (row HD) via fast recip + gpsimd bcast
                am = patt.tile([128, QT], bf16, name=f"am{pg}{qi}", tag="am")
                att_m[(pg, qi)] = am
                srow = prr.tile([1, 2 * QT], f32, name=f"sr{pg}{qi}", tag="sr")
                nc.vector.tensor_copy(srow[:], ao[HD:HD + 1, :])
                nc.vector.reciprocal_approx_fast(srow[:], srow[:])
                rb = pr.tile([HD, 2 * QT], f32, name=f"rb{pg}{qi}", tag="r")
                nc.gpsimd.partition_broadcast(rb[:], srow[:], channels=HD)
                nc.vector.tensor_mul(am[0:64, :], ao[0:HD, 0:QT], rb[:, 0:QT])
                nc.vector.tensor_mul(am[64:128, :], ao[0:HD, QT:2 * QT],
                                     rb[:, QT:2 * QT])

            with tc.tile_pool(name="xw", bufs=1) as xw:
                x_t = []
                wv_t = []
                wqk_t = []
                for kk in range(NDK):
                    r0 = slice(kk * 128, (kk + 1) * 128)
                    wv = xw.tile([128, F], bf16, name=f"wv{kk}", tag=f"wv{kk}")
                    nc.sync.dma_start(wv[:], w_sl[r0, 2 * F:3 * F])
                    wv_t.append(wv)
                    x_t.append(xw.tile([128, S], bf16, name=f"x{kk}",
                                       tag=f"x{kk}"))
                # x token-halves split across the scalar/gpsimd queues so
                # descriptor generation for x, wv and wq runs in parallel
                for ch in range(4):
                    cs = slice(ch * QT, (ch + 1) * QT)
                    xq = nc.scalar if ch < 2 else nc.gpsimd
                    for kk in range(NDK):
                        r0 = slice(kk * 128, (kk + 1) * 128)
                        xq.dma_start(x_t[kk][:, cs], xT[r0, cs])
                for kk in range(NDK):
                    r0 = slice(kk * 128, (kk + 1) * 128)
                    wq = xw.tile([128, 2 * F], bf16, name=f"wq{kk}",
                                 tag=f"wq{kk}")
                    nc.sync.dma_start(wq[:], w_sl[r0, 0:2 * F])
                    wqk_t.append(wq)
                for g in range(4):
                    nc.gpsimd.dma_start(wo_t[g][:],
                                        wo_sl[g * 128:(g + 1) * 128, :])

                def v_group(t2):
                    ps = psum.tile([128, 2 * QT], f32, name=f"pv{t2}",
                                   tag="big")
                    for kk in range(NDK):
                        for j in range(2):
                            tt = 2 * t2 + j
                            nc.tensor.matmul(
                                ps[:, j * F:j * F + F],
                                x_t[kk][:, tt * 128:(tt + 1) * 128],
                                wv_t[kk][:],
                                start=(kk == 0), stop=(kk == NDK - 1))
                    for j in range(2):
                        tt = 2 * t2 + j
                        vv = v_sb[tt].rearrange("p (h c) -> p h c", h=HPC)
                        pp = ps[:, j * F:j * F + F].rearrange(
                            "p (h c) -> p h c", h=HPC)
                        nc.vector.tensor_copy(vv[:, :, 0:HD], pp[:])
                        nc.vector.memset(vv[:, :, HD:HD + 1], 1.0)

                def qk_part(g, i):
                    part, th = i // 2, i % 2
                    dest = q_sb if part == 0 else k_sb
                    fcol = part * F + g * 128
                    ps = psum.tile([128, 2 * QT], f32,
                                   name=f"pq{part}{g}{th}", tag="big")
                    for kk in range(NDK):
                        for j in range(2):
                            tg = 2 * th + j
                            nc.tensor.matmul(
                                ps[:, j * QT:(j + 1) * QT],
                                wqk_t[kk][:, fcol:fcol + 128],
                                x_t[kk][:, tg * QT:(tg + 1) * QT],
                                start=(kk == 0), stop=(kk == NDK - 1))
                    nc.vector.tensor_copy(
                        dest[g][:, th * 2 * QT:(th + 1) * 2 * QT], ps[:])

                # v groups and next-pair qk quarters interleave with the
                # attention blocks so PE always has projection work while
                # ACT chews exps, and exps start as early as possible.
                # attn(0,0) only needs the token-half-0 q/k parts (i=0,2),
                # so the half-1 parts ride behind it.
                v_group(0)
                v_group(1)
                qk_part(0, 0)
                qk_part(0, 2)
                attn_block(0, 0)
                qk_part(0, 1)
                qk_part(0, 3)
                v_group(2)
                v_group(3)
                qk_part(1, 0)
                attn_block(0, 1)
                v_group(4)
                v_group(5)
                qk_part(1, 1)
                attn_block(0, 2)
                v_group(6)
                v_group(7)
                qk_part(1, 2)
                attn_block(0, 3)
                qk_part(1, 3)
                for g in range(1, 4):
                    for qi in range(NQI):
                        attn_block(g, qi)
                        if g < 3:
                            qk_part(g + 1, qi)
                        else:
                            opq.extend(op_unit(qi, dt) for dt in range(8))

            # ---- leftover out-proj units not absorbed by the g=3
            # attention blocks ----
            while opq:
                opq.pop(0)()

    nc.compile()
    return nc


def _get_nc():
    if "nc" not in _CACHE:
        _CACHE["nc"] = _build()
    return _CACHE["nc"]


def _prep_inputs(x, w_qkv, w_out, b_out):
    """Build the 8 per-core input maps (all payloads bf16)."""
    bf = ml_dtypes.bfloat16
    x = np.asarray(x, dtype=np.float32)
    w_qkv = np.asarray(w_qkv, dtype=np.float32)
    w_out = np.asarray(w_out, dtype=np.float32)

    tri = np.triu(np.ones((128, 128), dtype=np.float32))
    mask2 = np.tile(tri, (1, 2)).astype(bf)

    in_maps = []
    for c in range(8):
        b, hg = c // 2, c % 2
        cols = hg * F
        w_cat = np.concatenate([
            w_qkv[:, cols:cols + F],
            w_qkv[:, D + cols:D + cols + F],
            w_qkv[:, 2 * D + cols:2 * D + cols + F],
        ], axis=1)
        in_maps.append({
            "xT": np.ascontiguousarray(x[b].T).astype(bf),
            "w_sl": np.ascontiguousarray(w_cat).astype(bf),
            "wo_sl": np.ascontiguousarray(w_out[cols:cols + F, :]).astype(bf),
            "mask2": mask2,
        })
    return in_maps


def _run(inputs, trace=False):
    from concourse.bass_utils import run_bass_kernel_spmd

    nc = _get_nc()
    in_maps = _prep_inputs(**inputs)
    res = run_bass_kernel_spmd(nc, in_maps, core_ids=list(range(8)),
                               trace=trace)
    b_out = np.asarray(inputs["b_out"], dtype=np.float32)
    outs = []
    for b in range(B):
        o = (res.results[2 * b]["out"].astype(np.float32)
             + res.results[2 * b + 1]["out"].astype(np.float32))
        outs.append(o.T + b_out)
    full = np.stack(outs).astype(np.float32)
    return full, res


def kernel(x, w_qkv, w_out, b_out):
    full, _ = _run({"x": x, "w_qkv": w_qkv, "w_out": w_out, "b_out": b_out})
    return full
